# revision 14
# baseline (speedup 1.0000x reference)
"""Trainium2 Bass kernel for nn_BufferClassifier (B=32768, BUF=4096, H=10, T=10).

Strategy (pure data parallel over 8 NeuronCores, 4096 batch rows per core):
  - All rank-1 static branches (embeddings via one-hots, six Linear(1,10),
    Linear(11,10), biases) are folded on the host directly into the GEMM1
    weight: w_eff = [Ws @ w_hidden[:, :90].T ; w_hidden[:, 90:].T] with only
    25 raw input rows + 50 RNN-state rows = 75 contraction rows -> ONE
    matmul per 128-feature chunk.
  - The 5 Elman RNNs run as one fused width-50 recurrence (block-diagonal
    [50,50] hidden matmul + [5,50] input matmul + tanh w/ per-partition
    bias). 4 history branches share weights.
  - GEMM2 (hid @ w_ff) runs in bf16 (both operands) streaming w_ff from
    HBM; b_ff is folded into the softmax normalization as a precomputed
    exp(b_ff) per-class multiplier (saves one K=1 bias matmul per psum
    tile); softmax = exp on ACT, scale on DVE.
  - Software pipelining: the feature phase (RNN + GEMM1) of sub-tile s+1 is
    emitted interleaved into GEMM2(s)'s matmul stream (hid double-buffered),
    so the PE runs back-to-back matmuls throughout.
Batch is processed in sub-tiles of BSUB=512 columns (8 per core).
"""
import numpy as np

import concourse.bass as bass  # noqa: F401  (registers AP types)
from concourse import bacc
import concourse.mybir as mybir
import concourse.tile as tile

B = 32768
BUF = 4096
H = 10
T = 10
NCORES = 8
BC = B // NCORES            # rows per core
BSUB = 512                  # batch sub-tile (columns)
NSUB = BC // BSUB           # 8 sub-tiles per core
NM = BSUB // 128            # 4 output row-chunks per sub-tile
NK = BUF // 128             # 32 hid feature chunks
NCLS = BUF // 512           # 8 class chunks
NF = 75                     # fused feature rows (25 raw + 50 rnn)

F32R = mybir.dt.float32r
F32 = mybir.dt.float32
BF16 = mybir.dt.bfloat16
AF = mybir.ActivationFunctionType
AX = mybir.AxisListType

_CACHE = {}


def _build(nrep=1, abl=None):
    nc = bacc.Bacc(None, target_bir_lowering=False)
    d_rs = nc.dram_tensor("rs", [NSUB, 25, BSUB], F32R, kind="ExternalInput")
    d_rx = nc.dram_tensor("rx", [NSUB, 5, T, BSUB], F32R, kind="ExternalInput")
    d_Wrx = nc.dram_tensor("Wrx", [5, 50], F32R, kind="ExternalInput")
    d_Wrh = nc.dram_tensor("Wrh", [50, 50], F32R, kind="ExternalInput")
    d_rb = nc.dram_tensor("rb", [50, 1], F32, kind="ExternalInput")
    d_wc = nc.dram_tensor("wc", [NF, NK, 128], F32R, kind="ExternalInput")
    d_bh = nc.dram_tensor("bh", [128, NK], F32, kind="ExternalInput")
    d_wff = nc.dram_tensor("wff", [NCLS, NK // 4, 128, 4, 512], BF16,
                           kind="ExternalInput")
    d_expb = nc.dram_tensor("expb", [128, BUF], F32, kind="ExternalInput")
    d_out = nc.dram_tensor("out", [BC, BUF], F32, kind="ExternalOutput")

    with tile.TileContext(nc) as tc:
        with tc.tile_pool(name="const", bufs=1) as cst, \
             tc.tile_pool(name="hidp", bufs=2) as hidp, \
             tc.tile_pool(name="featp", bufs=2) as featp, \
             tc.tile_pool(name="hp", bufs=2) as hp, \
             tc.tile_pool(name="rxp", bufs=4) as rxp, \
             tc.tile_pool(name="wffp", bufs=3) as wffp, \
             tc.tile_pool(name="expp", bufs=1) as expp, \
             tc.tile_pool(name="smp", bufs=2) as smp, \
             tc.tile_pool(name="ps_aux", bufs=2, space="PSUM") as ps_aux, \
             tc.tile_pool(name="ps_g1", bufs=2, space="PSUM") as ps_g1, \
             tc.tile_pool(name="ps_g2", bufs=1, space="PSUM") as ps_g2:

            # --- constants, loaded once ---
            Wrxt = cst.tile([5, 50], F32R, name="Wrxt")
            nc.sync.dma_start(out=Wrxt, in_=d_Wrx[:, :])
            Wrht = cst.tile([50, 50], F32R, name="Wrht")
            nc.sync.dma_start(out=Wrht, in_=d_Wrh[:, :])
            rbt = cst.tile([50, 1], F32, name="rbt")
            nc.sync.dma_start(out=rbt, in_=d_rb[:, :])
            wct = cst.tile([NF, NK, 128], F32R, name="wct")
            nc.sync.dma_start(out=wct, in_=d_wc[:, :, :])
            bht = cst.tile([128, NK], F32, name="bht")
            nc.sync.dma_start(out=bht, in_=d_bh[:, :])
            expbt = cst.tile([128, BUF], F32, name="expbt")
            nc.sync.dma_start(out=expbt, in_=d_expb[:, :])

            exps = [expp.tile([128, BUF], F32, tag=f"exp{m}", name=f"exp{m}")
                    for m in range(NM)]

            wf_res = None
            if abl == "nodma":
                wf_res = wffp.tile([128, 4, 512], BF16, tag="wff",
                                   name="wf_only")
                nc.sync.dma_start(out=wf_res, in_=d_wff[0, 0])

            def feat_units(s):
                """Emission units for the feature phase of sub-tile s.
                Returns (units, hid_tiles); units[i] = (slot, fn)."""
                hid_s = [hidp.tile([128, BSUB], BF16, tag=f"hid{j}",
                                   name=f"hid_{s}_{j}") for j in range(NK)]
                feat = featp.tile([NF, BSUB], F32R, tag="feat",
                                  name=f"feat_{s}")
                st = {}
                units = []

                # feat rows: [0:50] = RNN final states (tanh writes at
                # partition base 0), [50:75] = raw static rows (DMA).
                def u_rs():
                    nc.sync.dma_start(out=feat[50:75, :], in_=d_rs[s])
                units.append((0, u_rs))

                def u_rxdma(t):
                    rxt = rxp.tile([5, BSUB], F32R, tag="rxt",
                                   name=f"rxt_{s}_{t}")
                    st[f"rx{t}"] = rxt
                    nc.sync.dma_start(out=rxt, in_=d_rx[s, :, t, :])

                def u_step(t):
                    ph = ps_aux.tile([50, BSUB], F32, tag="psx",
                                     name=f"ph_{s}_{t}")
                    nc.tensor.matmul(out=ph, lhsT=Wrxt[:], rhs=st[f"rx{t}"],
                                     start=True, stop=(t == 0))
                    if t > 0:
                        nc.tensor.matmul(out=ph, lhsT=Wrht[:],
                                         rhs=st["h"][:], start=False,
                                         stop=True)
                    if t == T - 1:
                        dst = feat[0:50, :]
                    else:
                        dst = hp.tile([50, BSUB], F32R, tag="h",
                                      name=f"h_{s}_{t}")
                    nc.scalar.activation(out=dst, in_=ph, func=AF.Tanh,
                                         bias=rbt[:])
                    st["h"] = dst

                for t in range(T):
                    units.append((max(0, 2 * t - 3), lambda t=t: u_rxdma(t)))
                    units.append((2 * t, lambda t=t: u_step(t)))

                def u_g1(j):
                    pg = ps_g1.tile([128, BSUB], F32, tag="psg1",
                                    name=f"pg_{s}_{j}")
                    nc.tensor.matmul(out=pg, lhsT=wct[:, j, :], rhs=feat[:],
                                     start=True, stop=True)
                    nc.scalar.activation(out=hid_s[j], in_=pg, func=AF.Relu,
                                         bias=bht[:, j:j + 1])

                for j in range(NK):
                    units.append((20 + j, lambda j=j: u_g1(j)))
                return units, hid_s

            def gemm2(s, hid_s, next_units):
                """GEMM2 + softmax for sub-tile s, with next feature-phase
                units interleaved at kq-slot granularity (64 slots)."""
                sched = {}
                for slot, fn in next_units:
                    sched.setdefault(min(slot, 63), []).append(fn)
                slot = 0
                for c in range(NCLS):
                    pts = [ps_g2.tile([128, 512], F32, tag=f"psg2_{m}",
                                      name=f"pt_{s}_{c}_{m}")
                           for m in range(NM)]
                    for kq in range(NK // 4):
                        if wf_res is not None:
                            wf = wf_res
                        else:
                            wf = wffp.tile([128, 4, 512], BF16, tag="wff",
                                           name=f"wf_{s}_{c}_{kq}")
                            nc.sync.dma_start(out=wf, in_=d_wff[c, kq])
                        for i in range(4):
                            k = kq * 4 + i
                            for m in range(NM):
                                nc.tensor.matmul(
                                    out=pts[m],
                                    lhsT=hid_s[k][:, m * 128:(m + 1) * 128],
                                    rhs=wf[:, i, :], start=(k == 0),
                                    stop=(k == NK - 1))
                        for fn in sched.pop(slot, ()):
                            fn()
                        slot += 1
                    for m in range(NM):
                        nc.scalar.activation(
                            out=exps[m][:, c * 512:(c + 1) * 512],
                            in_=pts[m], func=AF.Exp)
                for m in range(NM):
                    nc.vector.tensor_tensor(out=exps[m][:], in0=exps[m][:],
                                            in1=expbt[:],
                                            op=mybir.AluOpType.mult)
                    sm = smp.tile([128, 1], F32, tag=f"sum{m}",
                                  name=f"sum_{s}_{m}")
                    nc.vector.reduce_sum(out=sm, in_=exps[m][:], axis=AX.X)
                    rec = smp.tile([128, 1], F32, tag=f"rec{m}",
                                   name=f"rec_{s}_{m}")
                    nc.vector.reciprocal(rec, sm)
                    nc.vector.tensor_scalar(
                        out=exps[m][:], in0=exps[m][:], scalar1=rec[:],
                        scalar2=None, op0=mybir.AluOpType.mult)
                    row0 = s * BSUB + m * 128
                    nc.sync.dma_start(out=d_out[row0:row0 + 128, :],
                                      in_=exps[m][:])

            def body():
                units, hid_s = feat_units(0)
                for _, fn in units:
                    fn()
                for s in range(NSUB):
                    if s + 1 < NSUB:
                        next_units, next_hid = feat_units(s + 1)
                    else:
                        next_units, next_hid = [], None
                    gemm2(s, hid_s, next_units)
                    hid_s = next_hid

            if nrep == 1:
                body()
            else:
                with tc.For_i(0, nrep, 1):
                    body()
    nc.finalize()
    return nc


def _prep(inputs, g2bf=True):
    f = np.float32
    inputs = {k: np.asarray(v, f) for k, v in inputs.items()}
    data = inputs["data"]
    idx1 = data[:, 1].astype(np.int32)
    idx2 = data[:, 2].astype(np.int32)

    rs = np.empty((25, B), f)
    for r in range(3):
        rs[r] = (idx1 == r)
    for r in range(4):
        rs[3 + r] = (idx2 == r)
    rs[7:24] = data[:, 3:20].T
    rs[24] = 1.0

    # static-branch weights in feature-major layout [25, 90]
    Ws = np.zeros((25, 90), f)
    Ws[0:3, 0:10] = inputs["emb_client"]
    Ws[3:7, 10:20] = inputs["emb_lastreq"]
    for i, nm in enumerate(["req", "seq", "tac", "tcl", "tl"]):
        Ws[7 + i, 20 + 10 * i:30 + 10 * i] = inputs[f"w_{nm}"][:, 0]
        Ws[24, 20 + 10 * i:30 + 10 * i] = inputs[f"b_{nm}"]
    Ws[12:23, 70:80] = inputs["w_mem"].T
    Ws[24, 70:80] = inputs["b_mem"]
    Ws[23, 80:90] = inputs["w_cpu"][:, 0]
    Ws[24, 80:90] = inputs["b_cpu"]

    rx = np.ascontiguousarray(
        data[:, 20:70].reshape(B, 5, T).transpose(1, 2, 0))  # [5, T, B]

    wih = [inputs["pw_wih"]] + [inputs["h_wih"]] * 4
    whh = [inputs["pw_whh"]] + [inputs["h_whh"]] * 4
    bi = [inputs["pw_bih"] + inputs["pw_bhh"]] + \
         [inputs["h_bih"] + inputs["h_bhh"]] * 4
    Wrx = np.zeros((5, 50), f)
    Wrh = np.zeros((50, 50), f)
    for j in range(5):
        Wrx[j, 10 * j:10 * j + 10] = wih[j][:, 0]
        Wrh[10 * j:10 * j + 10, 10 * j:10 * j + 10] = whh[j].T
    rb = np.concatenate(bi).astype(f).reshape(50, 1)

    wh = np.ascontiguousarray(inputs["w_hidden"].T)       # [140, 4096]
    # fold the static-branch linear map into GEMM1: [25,90] @ [90,4096].
    # feat row order: [0:50] rnn states, [50:75] raw static rows.
    wc = np.concatenate([wh[90:140], Ws @ wh[0:90]], axis=0)  # [75, 4096]
    wc = np.ascontiguousarray(wc.reshape(NF, NK, 128))
    bh = np.ascontiguousarray(inputs["b_hidden"].reshape(NK, 128).T)

    import ml_dtypes
    wt = np.ascontiguousarray(inputs["w_ff"].T)           # [4096 feat, 4096 cls]
    wff = np.ascontiguousarray(
        wt.reshape(NK // 4, 4, 128, NCLS, 512).transpose(3, 0, 2, 1, 4)
    ).astype(ml_dtypes.bfloat16)
    expb = np.ascontiguousarray(np.broadcast_to(
        np.exp(inputs["b_ff"]).astype(f)[None, :], (128, BUF)))

    shared = dict(Wrx=Wrx, Wrh=Wrh, rb=rb, wc=wc, bh=bh, wff=wff, expb=expb)
    in_maps = []
    for c in range(NCORES):
        sl = slice(c * BC, (c + 1) * BC)
        rs_c = np.ascontiguousarray(
            rs[:, sl].reshape(25, NSUB, BSUB).transpose(1, 0, 2))
        rx_c = np.ascontiguousarray(
            rx[:, :, sl].reshape(5, T, NSUB, BSUB).transpose(2, 0, 1, 3))
        in_maps.append(dict(rs=rs_c, rx=rx_c, **shared))
    return in_maps


def get_nc(nrep=1, g2bf=True, g2ldw=False, abl=None):
    key = (nrep, abl)
    if key not in _CACHE:
        _CACHE[key] = _build(nrep, abl)
    return _CACHE[key]


def kernel(**inputs) -> np.ndarray:
    from concourse.bass_utils import run_bass_kernel_spmd
    nc = get_nc()
    in_maps = _prep(inputs)
    last = None
    for attempt in range(4):
        try:
            res = run_bass_kernel_spmd(nc, in_maps, core_ids=list(range(NCORES)))
            break
        except Exception as e:  # transient NRT device errors recover on retry
            last = e
            import time
            time.sleep(5 * (attempt + 1))
    else:
        raise last
    return np.concatenate([res.results[c]["out"] for c in range(NCORES)], axis=0)


# revision 26
# speedup vs baseline: 1.0096x; 1.0096x over previous
"""Trainium2 Bass kernel for nn_BufferClassifier (B=32768, BUF=4096, H=10, T=10).

Strategy (pure data parallel over 8 NeuronCores, 4096 batch rows per core):
  - All rank-1 static branches (embeddings via one-hots, six Linear(1,10),
    Linear(11,10), biases) are folded on the host directly into the GEMM1
    weight: w_eff = [Ws @ w_hidden[:, :90].T ; w_hidden[:, 90:].T] with only
    25 raw input rows + 50 RNN-state rows = 75 contraction rows -> ONE
    matmul per 128-feature chunk.
  - The 5 Elman RNNs run as one fused width-50 recurrence (block-diagonal
    [50,50] hidden matmul + [5,50] input matmul + tanh w/ per-partition
    bias). 4 history branches share weights.
  - GEMM2 (hid @ w_ff) runs in bf16 (both operands) streaming w_ff from
    HBM; b_ff is folded into the softmax normalization as a precomputed
    exp(b_ff) per-class multiplier (saves one K=1 bias matmul per psum
    tile); softmax = exp on ACT, scale on DVE.
  - Software pipelining: the feature phase (RNN + GEMM1) of sub-tile s+1 is
    emitted interleaved into GEMM2(s)'s matmul stream (hid double-buffered),
    so the PE runs back-to-back matmuls throughout.
Batch is processed in sub-tiles of BSUB=512 columns (8 per core).
"""
import numpy as np

import concourse.bass as bass  # noqa: F401  (registers AP types)
from concourse import bacc
import concourse.mybir as mybir
import concourse.tile as tile

B = 32768
BUF = 4096
H = 10
T = 10
NCORES = 8
BC = B // NCORES            # rows per core
BSUB = 512                  # batch sub-tile (columns)
NSUB = BC // BSUB           # 8 sub-tiles per core
NM = BSUB // 128            # 4 output row-chunks per sub-tile
NK = BUF // 128             # 32 hid feature chunks
NCLS = BUF // 512           # 8 class chunks
NF = 75                     # fused feature rows (25 raw + 50 rnn)

F32R = mybir.dt.float32r
F32 = mybir.dt.float32
BF16 = mybir.dt.bfloat16
AF = mybir.ActivationFunctionType
AX = mybir.AxisListType

_CACHE = {}


def _build(nrep=1, abl=None):
    nc = bacc.Bacc(None, target_bir_lowering=False)
    d_rs = nc.dram_tensor("rs", [NSUB, 25, BSUB], F32R, kind="ExternalInput")
    d_rx = nc.dram_tensor("rx", [NSUB, 5, T, BSUB], F32R, kind="ExternalInput")
    d_Wxh = nc.dram_tensor("Wxh", [55, 50], F32R, kind="ExternalInput")
    d_Wrx = nc.dram_tensor("Wrx", [5, 50], F32R, kind="ExternalInput")
    d_rb = nc.dram_tensor("rb", [50, 1], F32, kind="ExternalInput")
    d_wc = nc.dram_tensor("wc", [NF, NK, 128], F32R, kind="ExternalInput")
    d_bh = nc.dram_tensor("bh", [128, NK], F32, kind="ExternalInput")
    d_wff = nc.dram_tensor("wff", [NCLS, NK // 4, 128, 4, 512], BF16,
                           kind="ExternalInput")
    d_bffb = nc.dram_tensor("bffb", [128, BUF], F32, kind="ExternalInput")
    d_out = nc.dram_tensor("out", [BC, BUF], BF16, kind="ExternalOutput")

    with tile.TileContext(nc) as tc:
        with tc.tile_pool(name="const", bufs=1) as cst, \
             tc.tile_pool(name="hidp", bufs=2) as hidp, \
             tc.tile_pool(name="featp", bufs=2) as featp, \
             tc.tile_pool(name="hp", bufs=2) as hp, \
             tc.tile_pool(name="rxp", bufs=4) as rxp, \
             tc.tile_pool(name="wffp", bufs=3) as wffp, \
             tc.tile_pool(name="expp", bufs=2) as expp, \
             tc.tile_pool(name="smp", bufs=2) as smp, \
             tc.tile_pool(name="ps_aux", bufs=2, space="PSUM") as ps_aux, \
             tc.tile_pool(name="ps_g1", bufs=2, space="PSUM") as ps_g1, \
             tc.tile_pool(name="ps_g2", bufs=1, space="PSUM") as ps_g2:

            # --- constants, loaded once ---
            # Wxh rows [0:50] = Wrh (h part), [50:55] = Wrx (x part)
            Wxht = cst.tile([55, 50], F32R, name="Wxht")
            nc.sync.dma_start(out=Wxht, in_=d_Wxh[:, :])
            Wrxt = cst.tile([5, 50], F32R, name="Wrxt")
            nc.sync.dma_start(out=Wrxt, in_=d_Wrx[:, :])
            rbt = cst.tile([50, 1], F32, name="rbt")
            nc.sync.dma_start(out=rbt, in_=d_rb[:, :])
            wct = cst.tile([NF, NK, 128], F32R, name="wct")
            nc.sync.dma_start(out=wct, in_=d_wc[:, :, :])
            bht = cst.tile([128, NK], F32, name="bht")
            nc.sync.dma_start(out=bht, in_=d_bh[:, :])
            bffbt = cst.tile([128, BUF], F32, name="bffbt")
            nc.sync.dma_start(out=bffbt, in_=d_bffb[:, :])

            wf_res = None
            if abl == "nodma":
                wf_res = wffp.tile([128, 4, 512], BF16, tag="wff",
                                   name="wf_only")
                nc.sync.dma_start(out=wf_res, in_=d_wff[0, 0])

            def feat_units(s, head=False):
                """Emission units for the feature phase of sub-tile s.
                Returns (units, hid_tiles); units[i] = (slot, fn). With
                head=True (sub-tile 0), non-GEMM1 units get sentinel slot
                -100 (emitted immediately by body) and GEMM1 chunk j lands
                at slot j//4-1 so GEMM2(0) can start as soon as the first
                four hid chunks exist."""
                hid_s = [hidp.tile([128, BSUB], BF16, tag=f"hid{j}",
                                   name=f"hid_{s}_{j}") for j in range(NK)]
                feat = featp.tile([NF, BSUB], F32R, tag="feat",
                                  name=f"feat_{s}")
                st = {}
                units = []

                # feat rows: [0:50] = RNN final states (tanh writes at
                # partition base 0), [50:75] = raw static rows (DMA).
                def u_rs():
                    nc.sync.dma_start(out=feat[50:75, :], in_=d_rs[s])
                units.append((-100 if head else 0, u_rs))

                def u_rxdma(t):
                    # xh_t rows [0:50] = h_{t-1} (tanh of step t-1, base 0),
                    # rows [50:55] = x_t (DMA). Step 0 uses a separate tiny
                    # rx0 tile so its matmul operands sit at base 0.
                    if t == 0:
                        rx0 = hp.tile([5, BSUB], F32R, tag="rx0",
                                      name=f"rx0_{s}")
                        st["rx0"] = rx0
                        nc.sync.dma_start(out=rx0, in_=d_rx[s, :, 0, :])
                    else:
                        xh = hp.tile([55, BSUB], F32R, tag="xh",
                                     name=f"xh_{s}_{t}")
                        st[f"xh{t}"] = xh
                        nc.sync.dma_start(out=xh[50:55, :],
                                          in_=d_rx[s, :, t, :])

                def u_step(t):
                    ph = ps_aux.tile([50, BSUB], F32, tag="psx",
                                     name=f"ph_{s}_{t}")
                    if t == 0:
                        nc.tensor.matmul(out=ph, lhsT=Wrxt[:],
                                         rhs=st["rx0"][:],
                                         start=True, stop=True)
                    else:
                        nc.tensor.matmul(out=ph, lhsT=Wxht[:],
                                         rhs=st[f"xh{t}"][:],
                                         start=True, stop=True)
                    if t == T - 1:
                        dst = feat[0:50, :]
                    else:
                        dst = st[f"xh{t + 1}"][0:50, :]
                    nc.scalar.activation(out=dst, in_=ph, func=AF.Tanh,
                                         bias=rbt[:])

                # all rxdma units appended before step units so that xh_{t+1}
                # is allocated before step t (which writes h_t into it) at
                # equal slots.
                for t in range(T):
                    units.append((-100 if head else max(0, 2 * t - 4),
                                  lambda t=t: u_rxdma(t)))
                for t in range(T):
                    units.append((-100 if head else 2 * t,
                                  lambda t=t: u_step(t)))

                def u_g1(j):
                    pg = ps_g1.tile([128, BSUB], F32, tag="psg1",
                                    name=f"pg_{s}_{j}")
                    nc.tensor.matmul(out=pg, lhsT=wct[:, j, :], rhs=feat[:],
                                     start=True, stop=True)
                    nc.scalar.activation(out=hid_s[j], in_=pg, func=AF.Relu,
                                         bias=bht[:, j:j + 1])

                for j in range(NK):
                    units.append((j // 4 - 1 if head else 20 + j,
                                  lambda j=j: u_g1(j)))
                return units, hid_s

            def gemm2(s, hid_s, next_units):
                """GEMM2 + softmax for sub-tile s, with next feature-phase
                units interleaved at kq-slot granularity (64 slots)."""
                sched = {}
                for slot, fn in next_units:
                    sched.setdefault(min(slot, 63), []).append(fn)
                slot = 0
                exps = [expp.tile([128, BUF], BF16, tag=f"exp{m}",
                                  name=f"exp_{s}_{m}") for m in range(NM)]
                pars = [smp.tile([128, NCLS], F32, tag=f"par{m}",
                                 name=f"par_{s}_{m}") for m in range(NM)]
                for fn in sched.pop(-1, ()):
                    fn()
                for c in range(NCLS):
                    pts = [ps_g2.tile([128, 512], F32, tag=f"psg2_{m}",
                                      name=f"pt_{s}_{c}_{m}")
                           for m in range(NM)]
                    for kq in range(NK // 4):
                        if wf_res is not None:
                            wf = wf_res
                        else:
                            wf = wffp.tile([128, 4, 512], BF16, tag="wff",
                                           name=f"wf_{s}_{c}_{kq}")
                            nc.sync.dma_start(out=wf, in_=d_wff[c, kq])
                        for i in range(4):
                            k = kq * 4 + i
                            for m in range(NM):
                                nc.tensor.matmul(
                                    out=pts[m],
                                    lhsT=hid_s[k][:, m * 128:(m + 1) * 128],
                                    rhs=wf[:, i, :], start=(k == 0),
                                    stop=(k == NK - 1))
                        for fn in sched.pop(slot, ()):
                            fn()
                        slot += 1
                    for m in range(NM):
                        nc.vector.tensor_tensor(
                            out=pts[m], in0=pts[m],
                            in1=bffbt[:, c * 512:(c + 1) * 512],
                            op=mybir.AluOpType.add)
                        nc.scalar.activation(
                            out=exps[m][:, c * 512:(c + 1) * 512],
                            in_=pts[m], func=AF.Exp,
                            accum_out=pars[m][:, c:c + 1])
                for m in range(NM):
                    sm = smp.tile([128, 1], F32, tag=f"sum{m}",
                                  name=f"sum_{s}_{m}")
                    nc.vector.reduce_sum(out=sm, in_=pars[m][:], axis=AX.X)
                    rec = smp.tile([128, 1], F32, tag=f"rec{m}",
                                   name=f"rec_{s}_{m}")
                    nc.vector.reciprocal(rec, sm)
                    nc.vector.tensor_scalar(
                        out=exps[m][:], in0=exps[m][:], scalar1=rec[:],
                        scalar2=None, op0=mybir.AluOpType.mult)
                    row0 = s * BSUB + m * 128
                    nc.sync.dma_start(out=d_out[row0:row0 + 128, :],
                                      in_=exps[m][:])

            def body():
                units0, hid_s = feat_units(0, head=True)
                carry = []
                for slot, fn in units0:
                    if slot == -100:
                        fn()
                    else:
                        carry.append((slot, fn))
                for s in range(NSUB):
                    if s + 1 < NSUB:
                        next_units, next_hid = feat_units(s + 1)
                    else:
                        next_units, next_hid = [], None
                    gemm2(s, hid_s, carry + next_units)
                    carry = []
                    hid_s = next_hid

            if nrep == 1:
                body()
            else:
                with tc.For_i(0, nrep, 1):
                    body()
    nc.finalize()
    return nc


def _prep(inputs, g2bf=True):
    f = np.float32
    inputs = {k: np.asarray(v, f) for k, v in inputs.items()}
    data = inputs["data"]
    idx1 = data[:, 1].astype(np.int32)
    idx2 = data[:, 2].astype(np.int32)

    rs = np.empty((25, B), f)
    for r in range(3):
        rs[r] = (idx1 == r)
    for r in range(4):
        rs[3 + r] = (idx2 == r)
    rs[7:24] = data[:, 3:20].T
    rs[24] = 1.0

    # static-branch weights in feature-major layout [25, 90]
    Ws = np.zeros((25, 90), f)
    Ws[0:3, 0:10] = inputs["emb_client"]
    Ws[3:7, 10:20] = inputs["emb_lastreq"]
    for i, nm in enumerate(["req", "seq", "tac", "tcl", "tl"]):
        Ws[7 + i, 20 + 10 * i:30 + 10 * i] = inputs[f"w_{nm}"][:, 0]
        Ws[24, 20 + 10 * i:30 + 10 * i] = inputs[f"b_{nm}"]
    Ws[12:23, 70:80] = inputs["w_mem"].T
    Ws[24, 70:80] = inputs["b_mem"]
    Ws[23, 80:90] = inputs["w_cpu"][:, 0]
    Ws[24, 80:90] = inputs["b_cpu"]

    rx = np.ascontiguousarray(
        data[:, 20:70].reshape(B, 5, T).transpose(1, 2, 0))  # [5, T, B]

    wih = [inputs["pw_wih"]] + [inputs["h_wih"]] * 4
    whh = [inputs["pw_whh"]] + [inputs["h_whh"]] * 4
    bi = [inputs["pw_bih"] + inputs["pw_bhh"]] + \
         [inputs["h_bih"] + inputs["h_bhh"]] * 4
    Wxh = np.zeros((55, 50), f)
    for j in range(5):
        Wxh[10 * j:10 * j + 10, 10 * j:10 * j + 10] = whh[j].T
        Wxh[50 + j, 10 * j:10 * j + 10] = wih[j][:, 0]
    rb = np.concatenate(bi).astype(f).reshape(50, 1)

    wh = np.ascontiguousarray(inputs["w_hidden"].T)       # [140, 4096]
    # fold the static-branch linear map into GEMM1: [25,90] @ [90,4096].
    # feat row order: [0:50] rnn states, [50:75] raw static rows.
    wc = np.concatenate([wh[90:140], Ws @ wh[0:90]], axis=0)  # [75, 4096]
    wc = np.ascontiguousarray(wc.reshape(NF, NK, 128))
    bh = np.ascontiguousarray(inputs["b_hidden"].reshape(NK, 128).T)

    import ml_dtypes
    wt = np.ascontiguousarray(inputs["w_ff"].T)           # [4096 feat, 4096 cls]
    wff = np.ascontiguousarray(
        wt.reshape(NK // 4, 4, 128, NCLS, 512).transpose(3, 0, 2, 1, 4)
    ).astype(ml_dtypes.bfloat16)
    bffb = np.ascontiguousarray(np.broadcast_to(
        inputs["b_ff"].astype(f)[None, :], (128, BUF)))

    shared = dict(Wxh=Wxh, Wrx=Wxh[50:55].copy(), rb=rb, wc=wc, bh=bh,
                  wff=wff, bffb=bffb)
    in_maps = []
    for c in range(NCORES):
        sl = slice(c * BC, (c + 1) * BC)
        rs_c = np.ascontiguousarray(
            rs[:, sl].reshape(25, NSUB, BSUB).transpose(1, 0, 2))
        rx_c = np.ascontiguousarray(
            rx[:, :, sl].reshape(5, T, NSUB, BSUB).transpose(2, 0, 1, 3))
        in_maps.append(dict(rs=rs_c, rx=rx_c, **shared))
    return in_maps


def get_nc(nrep=1, g2bf=True, g2ldw=False, abl=None):
    key = (nrep, abl)
    if key not in _CACHE:
        _CACHE[key] = _build(nrep, abl)
    return _CACHE[key]


def kernel(**inputs) -> np.ndarray:
    from concourse.bass_utils import run_bass_kernel_spmd
    nc = get_nc()
    in_maps = _prep(inputs)
    last = None
    for attempt in range(4):
        try:
            res = run_bass_kernel_spmd(nc, in_maps, core_ids=list(range(NCORES)))
            break
        except Exception as e:  # transient NRT device errors recover on retry
            last = e
            import time
            time.sleep(5 * (attempt + 1))
    else:
        raise last
    return np.concatenate(
        [res.results[c]["out"].astype(np.float32) for c in range(NCORES)],
        axis=0)


# revision 29
# speedup vs baseline: 1.0257x; 1.0159x over previous
"""Trainium2 Bass kernel for nn_BufferClassifier (B=32768, BUF=4096, H=10, T=10).

Strategy (pure data parallel over 8 NeuronCores, 4096 batch rows per core):
  - All rank-1 static branches (embeddings via one-hots, six Linear(1,10),
    Linear(11,10), biases) are folded on the host directly into the GEMM1
    weight: w_eff = [Ws @ w_hidden[:, :90].T ; w_hidden[:, 90:].T] with only
    25 raw input rows + 50 RNN-state rows = 75 contraction rows -> ONE
    matmul per 128-feature chunk.
  - The 5 Elman RNNs run as one fused width-50 recurrence (block-diagonal
    [50,50] hidden matmul + [5,50] input matmul + tanh w/ per-partition
    bias). 4 history branches share weights.
  - GEMM2 (hid @ w_ff) runs in bf16 (both operands) streaming w_ff from
    HBM; b_ff is folded into the softmax normalization as a precomputed
    exp(b_ff) per-class multiplier (saves one K=1 bias matmul per psum
    tile); softmax = exp on ACT, scale on DVE.
  - Software pipelining: the feature phase (RNN + GEMM1) of sub-tile s+1 is
    emitted interleaved into GEMM2(s)'s matmul stream (hid double-buffered),
    so the PE runs back-to-back matmuls throughout.
Batch is processed in sub-tiles of BSUB=512 columns (8 per core).
"""
import numpy as np

import concourse.bass as bass  # noqa: F401  (registers AP types)
from concourse import bacc
import concourse.mybir as mybir
import concourse.tile as tile

B = 32768
BUF = 4096
H = 10
T = 10
NCORES = 8
BC = B // NCORES            # rows per core
BSUB = 512                  # batch sub-tile (columns)
NSUB = BC // BSUB           # 8 sub-tiles per core
NM = BSUB // 128            # 4 output row-chunks per sub-tile
NK = BUF // 128             # 32 hid feature chunks
NCLS = BUF // 512           # 8 class chunks
NF = 75                     # fused feature rows (25 raw + 50 rnn)

F32R = mybir.dt.float32r
F32 = mybir.dt.float32
BF16 = mybir.dt.bfloat16
AF = mybir.ActivationFunctionType
AX = mybir.AxisListType

_CACHE = {}


def _build(nrep=1, abl=None):
    nc = bacc.Bacc(None, target_bir_lowering=False)
    d_rs = nc.dram_tensor("rs", [NSUB, 25, BSUB], F32R, kind="ExternalInput")
    d_rx = nc.dram_tensor("rx", [NSUB, 5, T, BSUB], F32R, kind="ExternalInput")
    d_Wxh = nc.dram_tensor("Wxh", [55, 50], F32R, kind="ExternalInput")
    d_Wrx = nc.dram_tensor("Wrx", [5, 50], F32R, kind="ExternalInput")
    d_rb = nc.dram_tensor("rb", [50, 1], F32, kind="ExternalInput")
    d_wc = nc.dram_tensor("wc", [NF, NK, 128], F32R, kind="ExternalInput")
    d_bh = nc.dram_tensor("bh", [128, NK], F32, kind="ExternalInput")
    d_wff = nc.dram_tensor("wff", [NCLS, NK // 4, 128, 4, 512], BF16,
                           kind="ExternalInput")
    d_expb = nc.dram_tensor("expb", [128, BUF], BF16, kind="ExternalInput")
    d_out = nc.dram_tensor("out", [BC, BUF], BF16, kind="ExternalOutput")

    with tile.TileContext(nc) as tc:
        with tc.tile_pool(name="const", bufs=1) as cst, \
             tc.tile_pool(name="hidp", bufs=2) as hidp, \
             tc.tile_pool(name="featp", bufs=2) as featp, \
             tc.tile_pool(name="hp", bufs=2) as hp, \
             tc.tile_pool(name="rxp", bufs=4) as rxp, \
             tc.tile_pool(name="wffp", bufs=3) as wffp, \
             tc.tile_pool(name="expp", bufs=2) as expp, \
             tc.tile_pool(name="smp", bufs=2) as smp, \
             tc.tile_pool(name="ps_aux", bufs=2, space="PSUM") as ps_aux, \
             tc.tile_pool(name="ps_g1", bufs=2, space="PSUM") as ps_g1, \
             tc.tile_pool(name="ps_g2", bufs=1, space="PSUM") as ps_g2:

            # --- constants, loaded once ---
            # Wxh rows [0:50] = Wrh (h part), [50:55] = Wrx (x part)
            Wxht = cst.tile([55, 50], F32R, name="Wxht")
            nc.sync.dma_start(out=Wxht, in_=d_Wxh[:, :])
            Wrxt = cst.tile([5, 50], F32R, name="Wrxt")
            nc.sync.dma_start(out=Wrxt, in_=d_Wrx[:, :])
            rbt = cst.tile([50, 1], F32, name="rbt")
            nc.sync.dma_start(out=rbt, in_=d_rb[:, :])
            wct = cst.tile([NF, NK, 128], F32R, name="wct")
            nc.sync.dma_start(out=wct, in_=d_wc[:, :, :])
            bht = cst.tile([128, NK], F32, name="bht")
            nc.sync.dma_start(out=bht, in_=d_bh[:, :])
            expbt = cst.tile([128, BUF], BF16, name="expbt")
            nc.sync.dma_start(out=expbt, in_=d_expb[:, :])

            wf_res = None
            if abl == "nodma":
                wf_res = wffp.tile([128, 4, 512], BF16, tag="wff",
                                   name="wf_only")
                nc.sync.dma_start(out=wf_res, in_=d_wff[0, 0])

            def feat_units(s, head=False):
                """Emission units for the feature phase of sub-tile s.
                Returns (units, hid_tiles); units[i] = (slot, fn). With
                head=True (sub-tile 0), non-GEMM1 units get sentinel slot
                -100 (emitted immediately by body) and GEMM1 chunk j lands
                at slot j//4-1 so GEMM2(0) can start as soon as the first
                four hid chunks exist."""
                hid_s = [hidp.tile([128, BSUB], BF16, tag=f"hid{j}",
                                   name=f"hid_{s}_{j}") for j in range(NK)]
                feat = featp.tile([NF, BSUB], F32R, tag="feat",
                                  name=f"feat_{s}")
                st = {}
                units = []

                # feat rows: [0:50] = RNN final states (tanh writes at
                # partition base 0), [50:75] = raw static rows (DMA).
                def u_rs():
                    nc.sync.dma_start(out=feat[50:75, :], in_=d_rs[s])
                units.append((-100 if head else 0, u_rs))

                def u_rxdma(t):
                    # xh_t rows [0:50] = h_{t-1} (tanh of step t-1, base 0),
                    # rows [50:55] = x_t (DMA). Step 0 uses a separate tiny
                    # rx0 tile so its matmul operands sit at base 0.
                    if t == 0:
                        rx0 = hp.tile([5, BSUB], F32R, tag="rx0",
                                      name=f"rx0_{s}")
                        st["rx0"] = rx0
                        nc.sync.dma_start(out=rx0, in_=d_rx[s, :, 0, :])
                    else:
                        xh = hp.tile([55, BSUB], F32R, tag="xh",
                                     name=f"xh_{s}_{t}")
                        st[f"xh{t}"] = xh
                        nc.sync.dma_start(out=xh[50:55, :],
                                          in_=d_rx[s, :, t, :])

                def u_step(t):
                    ph = ps_aux.tile([50, BSUB], F32, tag="psx",
                                     name=f"ph_{s}_{t}")
                    if t == 0:
                        nc.tensor.matmul(out=ph, lhsT=Wrxt[:],
                                         rhs=st["rx0"][:],
                                         start=True, stop=True)
                    else:
                        nc.tensor.matmul(out=ph, lhsT=Wxht[:],
                                         rhs=st[f"xh{t}"][:],
                                         start=True, stop=True)
                    if t == T - 1:
                        dst = feat[0:50, :]
                    else:
                        dst = st[f"xh{t + 1}"][0:50, :]
                    nc.scalar.activation(out=dst, in_=ph, func=AF.Tanh,
                                         bias=rbt[:])

                # all rxdma units appended before step units so that xh_{t+1}
                # is allocated before step t (which writes h_t into it) at
                # equal slots.
                for t in range(T):
                    units.append((-100 if head else max(0, 2 * t - 4),
                                  lambda t=t: u_rxdma(t)))
                for t in range(T):
                    units.append((-100 if head else 2 * t,
                                  lambda t=t: u_step(t)))

                def u_g1(j):
                    pg = ps_g1.tile([128, BSUB], F32, tag="psg1",
                                    name=f"pg_{s}_{j}")
                    nc.tensor.matmul(out=pg, lhsT=wct[:, j, :], rhs=feat[:],
                                     start=True, stop=True)
                    nc.scalar.activation(out=hid_s[j], in_=pg, func=AF.Relu,
                                         bias=bht[:, j:j + 1])

                for j in range(NK):
                    units.append((j // 4 - 1 if head else 20 + j,
                                  lambda j=j: u_g1(j)))
                return units, hid_s

            def gemm2(s, hid_s, next_units):
                """GEMM2 + softmax for sub-tile s, with next feature-phase
                units interleaved at kq-slot granularity (64 slots)."""
                sched = {}
                for slot, fn in next_units:
                    sched.setdefault(min(slot, 63), []).append(fn)
                slot = 0
                exps = [expp.tile([128, BUF], BF16, tag=f"exp{m}",
                                  name=f"exp_{s}_{m}") for m in range(NM)]
                pars = [smp.tile([128, NCLS], F32, tag=f"par{m}",
                                 name=f"par_{s}_{m}") for m in range(NM)]
                for fn in sched.pop(-1, ()):
                    fn()
                for c in range(NCLS):
                    pts = [ps_g2.tile([128, 512], F32, tag=f"psg2_{m}",
                                      name=f"pt_{s}_{c}_{m}")
                           for m in range(NM)]
                    for kq in range(NK // 4):
                        if wf_res is not None:
                            wf = wf_res
                        else:
                            wf = wffp.tile([128, 4, 512], BF16, tag="wff",
                                           name=f"wf_{s}_{c}_{kq}")
                            nc.sync.dma_start(out=wf, in_=d_wff[c, kq])
                        for i in range(4):
                            k = kq * 4 + i
                            for m in range(NM):
                                nc.tensor.matmul(
                                    out=pts[m],
                                    lhsT=hid_s[k][:, m * 128:(m + 1) * 128],
                                    rhs=wf[:, i, :], start=(k == 0),
                                    stop=(k == NK - 1))
                        for fn in sched.pop(slot, ()):
                            fn()
                        slot += 1
                    for m in range(NM):
                        cs = slice(c * 512, (c + 1) * 512)
                        nc.scalar.activation(
                            out=exps[m][:, cs], in_=pts[m], func=AF.Exp)
                        # exps *= exp(b_ff); pars[:, c] = row-sum of product
                        nc.vector.tensor_tensor(
                            out=exps[m][:, cs], in0=exps[m][:, cs],
                            in1=expbt[:, cs], op=mybir.AluOpType.mult)
                        nc.vector.reduce_sum(out=pars[m][:, c:c + 1],
                                             in_=exps[m][:, cs], axis=AX.X)
                for m in range(NM):
                    sm = smp.tile([128, 1], F32, tag=f"sum{m}",
                                  name=f"sum_{s}_{m}")
                    nc.vector.reduce_sum(out=sm, in_=pars[m][:], axis=AX.X)
                    rec = smp.tile([128, 1], F32, tag=f"rec{m}",
                                   name=f"rec_{s}_{m}")
                    nc.vector.reciprocal(rec, sm)
                    nc.vector.tensor_scalar(
                        out=exps[m][:], in0=exps[m][:], scalar1=rec[:],
                        scalar2=None, op0=mybir.AluOpType.mult)
                    row0 = s * BSUB + m * 128
                    nc.sync.dma_start(out=d_out[row0:row0 + 128, :],
                                      in_=exps[m][:])

            def body():
                units0, hid_s = feat_units(0, head=True)
                carry = []
                for slot, fn in units0:
                    if slot == -100:
                        fn()
                    else:
                        carry.append((slot, fn))
                for s in range(NSUB):
                    if s + 1 < NSUB:
                        next_units, next_hid = feat_units(s + 1)
                    else:
                        next_units, next_hid = [], None
                    gemm2(s, hid_s, carry + next_units)
                    carry = []
                    hid_s = next_hid

            if nrep == 1:
                body()
            else:
                with tc.For_i(0, nrep, 1):
                    body()
    nc.finalize()
    return nc


def _prep(inputs, g2bf=True):
    f = np.float32
    inputs = {k: np.asarray(v, f) for k, v in inputs.items()}
    data = inputs["data"]
    idx1 = data[:, 1].astype(np.int32)
    idx2 = data[:, 2].astype(np.int32)

    rs = np.empty((25, B), f)
    for r in range(3):
        rs[r] = (idx1 == r)
    for r in range(4):
        rs[3 + r] = (idx2 == r)
    rs[7:24] = data[:, 3:20].T
    rs[24] = 1.0

    # static-branch weights in feature-major layout [25, 90]
    Ws = np.zeros((25, 90), f)
    Ws[0:3, 0:10] = inputs["emb_client"]
    Ws[3:7, 10:20] = inputs["emb_lastreq"]
    for i, nm in enumerate(["req", "seq", "tac", "tcl", "tl"]):
        Ws[7 + i, 20 + 10 * i:30 + 10 * i] = inputs[f"w_{nm}"][:, 0]
        Ws[24, 20 + 10 * i:30 + 10 * i] = inputs[f"b_{nm}"]
    Ws[12:23, 70:80] = inputs["w_mem"].T
    Ws[24, 70:80] = inputs["b_mem"]
    Ws[23, 80:90] = inputs["w_cpu"][:, 0]
    Ws[24, 80:90] = inputs["b_cpu"]

    rx = np.ascontiguousarray(
        data[:, 20:70].reshape(B, 5, T).transpose(1, 2, 0))  # [5, T, B]

    wih = [inputs["pw_wih"]] + [inputs["h_wih"]] * 4
    whh = [inputs["pw_whh"]] + [inputs["h_whh"]] * 4
    bi = [inputs["pw_bih"] + inputs["pw_bhh"]] + \
         [inputs["h_bih"] + inputs["h_bhh"]] * 4
    Wxh = np.zeros((55, 50), f)
    for j in range(5):
        Wxh[10 * j:10 * j + 10, 10 * j:10 * j + 10] = whh[j].T
        Wxh[50 + j, 10 * j:10 * j + 10] = wih[j][:, 0]
    rb = np.concatenate(bi).astype(f).reshape(50, 1)

    wh = np.ascontiguousarray(inputs["w_hidden"].T)       # [140, 4096]
    # fold the static-branch linear map into GEMM1: [25,90] @ [90,4096].
    # feat row order: [0:50] rnn states, [50:75] raw static rows.
    wc = np.concatenate([wh[90:140], Ws @ wh[0:90]], axis=0)  # [75, 4096]
    wc = np.ascontiguousarray(wc.reshape(NF, NK, 128))
    bh = np.ascontiguousarray(inputs["b_hidden"].reshape(NK, 128).T)

    import ml_dtypes
    wt = np.ascontiguousarray(inputs["w_ff"].T)           # [4096 feat, 4096 cls]
    wff = np.ascontiguousarray(
        wt.reshape(NK // 4, 4, 128, NCLS, 512).transpose(3, 0, 2, 1, 4)
    ).astype(ml_dtypes.bfloat16)
    import ml_dtypes as _mld
    expb = np.ascontiguousarray(np.broadcast_to(
        np.exp(inputs["b_ff"]).astype(_mld.bfloat16)[None, :], (128, BUF)))

    shared = dict(Wxh=Wxh, Wrx=Wxh[50:55].copy(), rb=rb, wc=wc, bh=bh,
                  wff=wff, expb=expb)
    in_maps = []
    for c in range(NCORES):
        sl = slice(c * BC, (c + 1) * BC)
        rs_c = np.ascontiguousarray(
            rs[:, sl].reshape(25, NSUB, BSUB).transpose(1, 0, 2))
        rx_c = np.ascontiguousarray(
            rx[:, :, sl].reshape(5, T, NSUB, BSUB).transpose(2, 0, 1, 3))
        in_maps.append(dict(rs=rs_c, rx=rx_c, **shared))
    return in_maps


def get_nc(nrep=1, g2bf=True, g2ldw=False, abl=None):
    key = (nrep, abl)
    if key not in _CACHE:
        _CACHE[key] = _build(nrep, abl)
    return _CACHE[key]


def kernel(**inputs) -> np.ndarray:
    from concourse.bass_utils import run_bass_kernel_spmd
    nc = get_nc()
    in_maps = _prep(inputs)
    last = None
    for attempt in range(4):
        try:
            res = run_bass_kernel_spmd(nc, in_maps, core_ids=list(range(NCORES)))
            break
        except Exception as e:  # transient NRT device errors recover on retry
            last = e
            import time
            time.sleep(5 * (attempt + 1))
    else:
        raise last
    return np.concatenate(
        [res.results[c]["out"].astype(np.float32) for c in range(NCORES)],
        axis=0)
